# revision 9
# baseline (speedup 1.0000x reference)
"""Trainium2 Bass kernel for nn_GCN_15590731285230 (v3.2).

Rig cost profile (microbenched 2026-08-09): gpsimd partition ops ~0.7ms EACH;
every dma_start ~1ms regardless of size; matmuls/small ops ~5-11us flat; big
[128, 32768] DVE/ACT passes ~150us. Design responses:

  * NO gpsimd: partition-direction max/sum via 16 PE transposes + one strided
    DVE reduce; [1,n]-row values broadcast to 128 partitions with
    selector-matrix matmuls (sel16[c,(j,q)] = ident[c,j], materialized by a
    single stride-0-read DVE copy).
  * 2 DMAs total: one packed input (x chunks + wr/w1/w2 pre-packed by
    kernel() into [128, 19, 128]) and the output store. The old per-row
    reassembly DMAs (mrow/srow/r0row) are replaced by selector matmuls
    straight off the [16, 128] stat tiles.
  * r0 normalization (1/s[0]) applied to the reduced v vector via
    stile_sb[:, 0:1] instead of scaling the r0 row.
  * All matmuls fp32 (self-loading weights; 16-bit would add Ldweights).

Per batch b (core b):
    R  = softmax(x wr x^T, axis=-1);  h1 = relu(R x w1 + x)
    out_b = relu(R[0,:] @ h1 @ w2 + h1[0,:])
"""

import sys

if "/opt/trn_rl_repo" not in sys.path:
    sys.path.insert(0, "/opt/trn_rl_repo")

from contextlib import ExitStack

import numpy as np

import concourse.bacc as bacc
import concourse.bass as bass
import concourse.mybir as mybir
import concourse.tile as tile
from concourse.bass_utils import run_bass_kernel_spmd
from concourse.masks import make_identity

P = 128
D = 128
B = 8
F32 = mybir.dt.float32
AF = mybir.ActivationFunctionType
AX = mybir.AxisListType
ALU = mybir.AluOpType


def _bcast_free(ap, count):
    """Insert a stride-0 dim of size `count` after the partition dim."""
    return bass.AP(tensor=ap.tensor, offset=ap.offset,
                   ap=[list(ap.ap[0]), [0, count]] + [list(d) for d in ap.ap[1:]])


def build_kernel(n=2048, repeat=1):
    nt = n // P              # m chunks
    w5 = min(512, n)         # matmul moving-operand width
    pair = 2 if nt % 2 == 0 else 1   # chunks per PSUM drain in pass A
    nc = bacc.Bacc()
    # packed input: [:, 0:nt, :] = x chunks (node c=t*128+p <-> row p*nt+t),
    # [:, nt+0/1/2, :] = wr/w1/w2 rows
    xall_d = nc.dram_tensor("xall", [P, nt + 3, D], F32, kind="ExternalInput")
    out_d = nc.dram_tensor("out", [1, D], F32, kind="ExternalOutput")

    with tile.TileContext(nc) as tc, ExitStack() as ctx:
        sg = ctx.enter_context(tc.tile_pool(name="sg", bufs=1))
        scr = ctx.enter_context(tc.tile_pool(name="scr", bufs=1))
        st = ctx.enter_context(tc.tile_pool(name="st", bufs=1))

        for _rep in range(repeat):
            ident = sg.tile([P, P], F32, tag="ident")
            make_identity(nc, ident)
            # selector: sel16[c, j, q] = ident[c, j]  (stride-0 read on q)
            sel16 = sg.tile([nt, nt, P], F32, tag="sel16")
            nc.vector.tensor_copy(
                sel16,
                bass.AP(tensor=ident.tensor, offset=ident.offset,
                        ap=[[ident.ap[0][0], nt], [1, nt], [0, P]]))

            xall = sg.tile([P, nt + 3, P], F32, tag="xall")
            nc.sync.dma_start(xall, xall_d[:])
            xnat = xall[:, 0:nt, :]
            wr_sb = xall[:, nt + 0, :]
            w1_sb = xall[:, nt + 1, :]
            w2_sb = xall[:, nt + 2, :]

            # xT via PE transposes packed into wide PSUM tensors; yT = (x wr)^T.
            xT = sg.tile([P, n], F32, tag="xT")
            yT = sg.tile([P, n], F32, tag="yT")
            with tc.tile_pool(name="pst", bufs=2, space="PSUM") as pst:
                half = max(n // 2, P)
                for h in range(0, n, half):
                    tp = pst.tile([P, half], F32, tag="tp")
                    for k in range(half // P):
                        nc.tensor.transpose(tp[:, k * P:(k + 1) * P],
                                            xnat[:, h // P + k, :], ident)
                    nc.vector.tensor_copy(xT[:, h:h + half], tp)
                wy = min(w5, half)
                for h in range(0, n, half):
                    yp = pst.tile([P, half], F32, tag="tp")
                    for j in range(0, half, wy):
                        nc.tensor.matmul(yp[:, j:j + wy], lhsT=wr_sb,
                                         rhs=xT[:, h + j:h + j + wy],
                                         start=True, stop=True)
                    nc.vector.tensor_copy(yT[:, h:h + half], yp)

            # ---- pass A: ST[m, n] = S[n, m], fp32 in SBUF ----
            stall = sg.tile([P, nt, n], F32, tag="stall")
            stall_flat = stall.rearrange("p t n -> p (t n)")
            with tc.tile_pool(name="psA", bufs=1, space="PSUM") as psA:
                for g in range(0, nt, pair):
                    sp = psA.tile([P, pair * n], F32, tag="sp")
                    for k in range(pair):
                        for j in range(0, n, w5):
                            nc.tensor.matmul(
                                sp[:, k * n + j:k * n + j + w5],
                                lhsT=xT[:, (g + k) * P:(g + k + 1) * P],
                                rhs=yT[:, j:j + w5],
                                start=True, stop=True)
                    nc.scalar.copy(
                        stall_flat[:, g * n:(g + pair) * n], sp)

            # ---- softmax stats: gpsimd-free, DMA-free ----
            mx_pt = scr.tile([P, n], F32, tag="scr")
            nc.vector.tensor_reduce(mx_pt, stall.rearrange("p t n -> p n t"),
                                    axis=AX.X, op=ALU.max)
            with tc.tile_pool(name="psS", bufs=1, space="PSUM") as psS:
                mxT = psS.tile([P, n], F32, tag="xt")
                for j in range(nt):
                    nc.tensor.transpose(mxT[:, j * P:(j + 1) * P],
                                        mx_pt[:, j * P:(j + 1) * P], ident)
                colmax = st.tile([P, nt], F32, tag="colmax")
                nc.vector.tensor_reduce(
                    colmax, mxT.rearrange("p (j m) -> p j m", j=nt),
                    axis=AX.X, op=ALU.max)
                cmT = psS.tile([nt, P], F32, tag="xt")  # mxT slot is free now
                nc.tensor.transpose(cmT, colmax, ident)
                cm16 = st.tile([nt, P], F32, tag="cm16")
                nc.vector.tensor_copy(cm16, cmT)
                # broadcast: mtile[q, j*128+p] = cm16[j, p] via selector mms
                mtile = psS.tile([P, n], F32, tag="bc")
                for j in range(nt):
                    nc.tensor.matmul(mtile[:, j * P:(j + 1) * P],
                                     lhsT=sel16[:, j, :], rhs=cm16,
                                     start=True, stop=True)

                # ---- softmax numerator: one sub + one in-place exp ----
                nc.vector.tensor_sub(stall_flat, stall_flat,
                                     _bcast_free(mtile[:], nt))
            nc.scalar.activation(stall_flat, stall_flat, AF.Exp)

            # column sums -> reciprocals, same structure
            et_pt = scr.tile([P, n], F32, tag="scr")
            nc.vector.tensor_reduce(et_pt, stall.rearrange("p t n -> p n t"),
                                    axis=AX.X, op=ALU.add)
            stile_sb = sg.tile([P, n], F32, tag="stile_sb")
            with tc.tile_pool(name="psS2", bufs=1, space="PSUM") as psS2:
                etT = psS2.tile([P, n], F32, tag="xt")
                for j in range(nt):
                    nc.tensor.transpose(etT[:, j * P:(j + 1) * P],
                                        et_pt[:, j * P:(j + 1) * P], ident)
                colsum = st.tile([P, nt], F32, tag="colmax")
                nc.vector.tensor_reduce(
                    colsum, etT.rearrange("p (j m) -> p j m", j=nt),
                    axis=AX.X, op=ALU.add)
                nc.vector.reciprocal(colsum, colsum)
                csT = psS2.tile([nt, P], F32, tag="xt")  # etT slot is free now
                nc.tensor.transpose(csT, colsum, ident)
                cs16 = st.tile([nt, P], F32, tag="cm16")
                nc.vector.tensor_copy(cs16, csT)
                stile = psS2.tile([P, n], F32, tag="bc")
                for j in range(nt):
                    nc.tensor.matmul(stile[:, j * P:(j + 1) * P],
                                     lhsT=sel16[:, j, :], rhs=cs16,
                                     start=True, stop=True)
                nc.scalar.copy(stile_sb, stile)     # keep past pool close

            # ---- Z^T accumulation over chunks (fp32, self-loading mms) ----
            with tc.tile_pool(name="psB", bufs=1, space="PSUM") as psB:
                ztp = psB.tile([P, n], F32, tag="zt")
                for t in range(nt):
                    for j in range(0, n, w5):
                        nc.tensor.matmul(ztp[:, j:j + w5],
                                         lhsT=xnat[:, t, :],
                                         rhs=stall[:, t, j:j + w5],
                                         start=(t == 0), stop=(t == nt - 1))
                # znorm = ZT * (1/s): PSUM->SBUF copy and scale in one op
                znorm = sg.tile([P, n], F32, tag="yT")   # reuses yT slot
                nc.vector.tensor_mul(znorm, ztp, stile_sb)

                # ---- h1T = relu(w1^T Znorm + xT) ----
                h1t = sg.tile([P, n], F32, tag="h1t")
                hp = psB.tile([P, n], F32, tag="hp")
                for j in range(0, n, w5):
                    nc.tensor.matmul(hp[:, j:j + w5], lhsT=w1_sb,
                                     rhs=znorm[:, j:j + w5],
                                     start=True, stop=True)
                nc.vector.tensor_add(h1t, hp, xT)
                nc.vector.tensor_relu(h1t, h1t)

                # ---- tail: out = relu(r0 @ h1 @ w2 + h1[0, :]) ----
                # r0 (unnormalized) = exp'd scores column n=0 = stall[:, :, 0]
                rtp = psB.tile([nt, P], F32, tag="zt")  # reuses ztp banks
                nc.tensor.transpose(
                    rtp, stall[:, :, 0:1].rearrange("p t o -> p (t o)"),
                    ident)
                r16 = st.tile([nt, P], F32, tag="r16")
                nc.vector.tensor_copy(r16, rtp)
                # broadcast unnormalized r0 via selector mms
                r0tile = psB.tile([P, n], F32, tag="hp")  # reuses hp banks
                for j in range(nt):
                    nc.tensor.matmul(r0tile[:, j * P:(j + 1) * P],
                                     lhsT=sel16[:, j, :], rhs=r16,
                                     start=True, stop=True)
                wsum = sg.tile([P, n], F32, tag="yT")    # reuses znorm slot
                nc.vector.tensor_mul(wsum, h1t, r0tile)
                v = st.tile([P, 1], F32, tag="v")
                nc.vector.tensor_reduce(v, wsum, axis=AX.X, op=ALU.add)
                # normalize by 1/s[0]: stile_sb column 0 holds it, broadcast
                nc.vector.tensor_mul(v, v, stile_sb[:, 0:1])
                o2 = psB.tile([1, P], F32, tag="zt")  # rtp/ztp banks are free
                nc.tensor.matmul(o2, lhsT=v, rhs=w2_sb, start=True, stop=False)
                nc.tensor.matmul(o2, lhsT=h1t[:, 0:1], rhs=ident,
                                 start=False, stop=True)
                fin = st.tile([1, P], F32, tag="fin")
                nc.scalar.activation(fin, o2, AF.Relu)
                nc.sync.dma_start(out_d[:], fin)

    nc.compile()
    return nc


_CACHE = {}


def pack_inputs(x, w1, w2, wr):
    """Pack one batch's x plus the three weights into [128, nt+3, 128]."""
    n, d = x.shape
    nt = n // P
    pk = np.empty((P, nt + 3, d), dtype=np.float32)
    pk[:, 0:nt, :] = x.reshape(P, nt, d)
    pk[:, nt + 0, :] = wr
    pk[:, nt + 1, :] = w1
    pk[:, nt + 2, :] = w2
    return np.ascontiguousarray(pk)


def kernel(x, w1, w2, wr):
    x = np.ascontiguousarray(np.asarray(x), dtype=np.float32)
    w1 = np.ascontiguousarray(np.asarray(w1), dtype=np.float32)
    w2 = np.ascontiguousarray(np.asarray(w2), dtype=np.float32)
    wr = np.ascontiguousarray(np.asarray(wr), dtype=np.float32)
    b, n, d = x.shape
    if "nc" not in _CACHE:
        _CACHE["nc"] = build_kernel(n)
    nc = _CACHE["nc"]
    in_maps = [{"xall": pack_inputs(x[i], w1, w2, wr)} for i in range(b)]
    res = run_bass_kernel_spmd(nc, in_maps, core_ids=list(range(b)))
    return np.stack([res.results[i]["out"][0] for i in range(b)])


# revision 10
# speedup vs baseline: 1.3545x; 1.3545x over previous
"""Trainium2 Bass kernel for nn_GCN_15590731285230 (v3.3).

Rig cost profile (microbenched 2026-08-09): gpsimd partition ops ~0.7ms EACH;
dma_start calls are expensive (fixed cost, size-independent); matmuls/small
ops ~10-60us flat; big [128, 32768] DVE/ACT passes ~150us. Design:

  * NO gpsimd. Column MAX of the score matrix: strided DVE chunk-reduce,
    16 PE transposes, strided DVE reduce -> [1,n] row via small DMA, then
    broadcast to 128 partitions with ones-row matmuls (contraction 1).
  * Column SUM is linear, so it skips transposes entirely: ones-column
    matmuls (lhsT = ones[128,1]) give true column sums [1, n] directly in
    PSUM; reciprocal lands in SBUF; ones-row matmuls broadcast 1/s.
  * ONE input DMA: kernel() pre-packs x chunks + wr/w1/w2 into
    [128, nt+3, 128] (node c = t*128+p <-> x row p*nt+t).
  * All matmuls fp32 (self-loading weights; 16-bit adds Ldweights = 2x).

Per batch b (core b):
    R  = softmax(x wr x^T, axis=-1);  h1 = relu(R x w1 + x)
    out_b = relu(R[0,:] @ h1 @ w2 + h1[0,:])
"""

import sys

if "/opt/trn_rl_repo" not in sys.path:
    sys.path.insert(0, "/opt/trn_rl_repo")

from contextlib import ExitStack

import numpy as np

import concourse.bacc as bacc
import concourse.bass as bass
import concourse.mybir as mybir
import concourse.tile as tile
from concourse.bass_utils import run_bass_kernel_spmd
from concourse.masks import make_identity

P = 128
D = 128
B = 8
F32 = mybir.dt.float32
AF = mybir.ActivationFunctionType
AX = mybir.AxisListType
ALU = mybir.AluOpType


def _bcast_free(ap, count):
    """Insert a stride-0 dim of size `count` after the partition dim."""
    return bass.AP(tensor=ap.tensor, offset=ap.offset,
                   ap=[list(ap.ap[0]), [0, count]] + [list(d) for d in ap.ap[1:]])


def build_kernel(n=2048, repeat=1):
    nt = n // P              # m chunks
    w5 = min(512, n)         # matmul moving-operand width
    pair = 2 if nt % 2 == 0 else 1   # chunks per PSUM drain in pass A
    nc = bacc.Bacc()
    xall_d = nc.dram_tensor("xall", [P, nt + 3, D], F32, kind="ExternalInput")
    out_d = nc.dram_tensor("out", [1, D], F32, kind="ExternalOutput")

    with tile.TileContext(nc) as tc, ExitStack() as ctx:
        sg = ctx.enter_context(tc.tile_pool(name="sg", bufs=1))
        scr = ctx.enter_context(tc.tile_pool(name="scr", bufs=1))
        st = ctx.enter_context(tc.tile_pool(name="st", bufs=1))

        for _rep in range(repeat):
            ident = sg.tile([P, P], F32, tag="ident")
            make_identity(nc, ident)
            ones1 = sg.tile([1, P], F32, tag="ones1")
            nc.vector.memset(ones1, 1.0)
            ones128 = sg.tile([P, 1], F32, tag="ones128")
            nc.vector.memset(ones128, 1.0)

            xall = sg.tile([P, nt + 3, P], F32, tag="xall")
            nc.sync.dma_start(xall, xall_d[:])
            xnat = xall[:, 0:nt, :]
            wr_sb = xall[:, nt + 0, :]
            w1_sb = xall[:, nt + 1, :]
            w2_sb = xall[:, nt + 2, :]

            # xT via PE transposes packed into wide PSUM tensors; yT = (x wr)^T.
            xT = sg.tile([P, n], F32, tag="xT")
            yT = sg.tile([P, n], F32, tag="yT")
            with tc.tile_pool(name="pst", bufs=2, space="PSUM") as pst:
                half = max(n // 2, P)
                for h in range(0, n, half):
                    tp = pst.tile([P, half], F32, tag="tp")
                    for k in range(half // P):
                        nc.tensor.transpose(tp[:, k * P:(k + 1) * P],
                                            xnat[:, h // P + k, :], ident)
                    nc.vector.tensor_copy(xT[:, h:h + half], tp)
                wy = min(w5, half)
                for h in range(0, n, half):
                    yp = pst.tile([P, half], F32, tag="tp")
                    for j in range(0, half, wy):
                        nc.tensor.matmul(yp[:, j:j + wy], lhsT=wr_sb,
                                         rhs=xT[:, h + j:h + j + wy],
                                         start=True, stop=True)
                    nc.vector.tensor_copy(yT[:, h:h + half], yp)

            # ---- pass A: ST[m, n] = S[n, m], fp32 in SBUF ----
            stall = sg.tile([P, nt, n], F32, tag="stall")
            stall_flat = stall.rearrange("p t n -> p (t n)")
            with tc.tile_pool(name="psA", bufs=1, space="PSUM") as psA:
                for g in range(0, nt, pair):
                    sp = psA.tile([P, pair * n], F32, tag="sp")
                    for k in range(pair):
                        for j in range(0, n, w5):
                            nc.tensor.matmul(
                                sp[:, k * n + j:k * n + j + w5],
                                lhsT=xT[:, (g + k) * P:(g + k + 1) * P],
                                rhs=yT[:, j:j + w5],
                                start=True, stop=True)
                    nc.scalar.copy(
                        stall_flat[:, g * n:(g + pair) * n], sp)

            # ---- column max over m (partitions x chunks), gpsimd-free ----
            mx_pt = scr.tile([P, n], F32, tag="scr")
            nc.vector.tensor_reduce(mx_pt, stall.rearrange("p t n -> p n t"),
                                    axis=AX.X, op=ALU.max)
            with tc.tile_pool(name="psS", bufs=1, space="PSUM") as psS:
                mxT = psS.tile([P, n], F32, tag="xt")
                for j in range(nt):
                    nc.tensor.transpose(mxT[:, j * P:(j + 1) * P],
                                        mx_pt[:, j * P:(j + 1) * P], ident)
                colmax = st.tile([P, nt], F32, tag="colmax")
                nc.vector.tensor_reduce(
                    colmax, mxT.rearrange("p (j m) -> p j m", j=nt),
                    axis=AX.X, op=ALU.max)
                cmT = psS.tile([nt, P], F32, tag="xt")  # mxT slot is free now
                nc.tensor.transpose(cmT, colmax, ident)
                cm16 = st.tile([nt, P], F32, tag="cm16")
                nc.vector.tensor_copy(cm16, cmT)
                mrow = scr.tile([1, n], F32, tag="mrow")
                nc.sync.dma_start(
                    mrow.rearrange("o (j p) -> o j p", j=nt), cm16)
                # broadcast to all 128 partitions via ones-row matmuls
                mtile = psS.tile([P, n], F32, tag="bc")
                for j in range(0, n, w5):
                    nc.tensor.matmul(mtile[:, j:j + w5], lhsT=ones1,
                                     rhs=mrow[:, j:j + w5],
                                     start=True, stop=True)

                # ---- softmax numerator: one sub + one in-place exp ----
                nc.vector.tensor_sub(stall_flat, stall_flat,
                                     _bcast_free(mtile[:], nt))
            nc.scalar.activation(stall_flat, stall_flat, AF.Exp)

            # ---- column sums: linear -> ones-column matmuls, no transposes
            et_pt = scr.tile([P, n], F32, tag="scr")
            nc.vector.tensor_reduce(et_pt, stall.rearrange("p t n -> p n t"),
                                    axis=AX.X, op=ALU.add)
            stile_sb = sg.tile([P, n], F32, tag="stile_sb")
            rrow = scr.tile([1, n], F32, tag="rrow")
            with tc.tile_pool(name="psS2", bufs=1, space="PSUM") as psS2:
                srow = psS2.tile([1, n], F32, tag="srw")
                for j in range(0, n, w5):
                    nc.tensor.matmul(srow[:, j:j + w5], lhsT=ones128,
                                     rhs=et_pt[:, j:j + w5],
                                     start=True, stop=True)
                nc.vector.reciprocal(rrow, srow)    # PSUM -> SBUF, 1/s row
                stile = psS2.tile([P, n], F32, tag="bc")
                for j in range(0, n, w5):
                    nc.tensor.matmul(stile[:, j:j + w5], lhsT=ones1,
                                     rhs=rrow[:, j:j + w5],
                                     start=True, stop=True)
                nc.scalar.copy(stile_sb, stile)     # keep past pool close

            # ---- Z^T accumulation over chunks (fp32, self-loading mms) ----
            with tc.tile_pool(name="psB", bufs=1, space="PSUM") as psB:
                ztp = psB.tile([P, n], F32, tag="zt")
                for t in range(nt):
                    for j in range(0, n, w5):
                        nc.tensor.matmul(ztp[:, j:j + w5],
                                         lhsT=xnat[:, t, :],
                                         rhs=stall[:, t, j:j + w5],
                                         start=(t == 0), stop=(t == nt - 1))
                # znorm = ZT * (1/s): PSUM->SBUF copy and scale in one op
                znorm = sg.tile([P, n], F32, tag="yT")   # reuses yT slot
                nc.vector.tensor_mul(znorm, ztp, stile_sb)

                # ---- h1T = relu(w1^T Znorm + xT) ----
                h1t = sg.tile([P, n], F32, tag="h1t")
                hp = psB.tile([P, n], F32, tag="hp")
                for j in range(0, n, w5):
                    nc.tensor.matmul(hp[:, j:j + w5], lhsT=w1_sb,
                                     rhs=znorm[:, j:j + w5],
                                     start=True, stop=True)
                nc.vector.tensor_add(h1t, hp, xT)
                nc.vector.tensor_relu(h1t, h1t)

                # ---- tail: out = relu(r0 @ h1 @ w2 + h1[0, :]) ----
                # r0 (unnormalized) = exp'd scores column n=0 = stall[:, :, 0]
                rtp = psB.tile([nt, P], F32, tag="zt")  # reuses ztp banks
                nc.tensor.transpose(
                    rtp, stall[:, :, 0:1].rearrange("p t o -> p (t o)"),
                    ident)
                r16 = st.tile([nt, P], F32, tag="r16")
                nc.vector.tensor_copy(r16, rtp)
                r0row = scr.tile([1, n], F32, tag="rrow2")
                nc.sync.dma_start(
                    r0row.rearrange("o (t p) -> o t p", t=nt), r16)
                # normalize by 1/s[0] (rrow holds reciprocals)
                nc.vector.tensor_scalar_mul(r0row, r0row, rrow[0:1, 0:1])
                r0tile = psB.tile([P, n], F32, tag="hp")  # reuses hp banks
                for j in range(0, n, w5):
                    nc.tensor.matmul(r0tile[:, j:j + w5], lhsT=ones1,
                                     rhs=r0row[:, j:j + w5],
                                     start=True, stop=True)
                wsum = sg.tile([P, n], F32, tag="yT")    # reuses znorm slot
                nc.vector.tensor_mul(wsum, h1t, r0tile)
                v = st.tile([P, 1], F32, tag="v")
                nc.vector.tensor_reduce(v, wsum, axis=AX.X, op=ALU.add)
                o2 = psB.tile([1, P], F32, tag="zt")  # rtp/ztp banks are free
                nc.tensor.matmul(o2, lhsT=v, rhs=w2_sb, start=True, stop=False)
                nc.tensor.matmul(o2, lhsT=h1t[:, 0:1], rhs=ident,
                                 start=False, stop=True)
                fin = st.tile([1, P], F32, tag="fin")
                nc.scalar.activation(fin, o2, AF.Relu)
                nc.sync.dma_start(out_d[:], fin)

    nc.compile()
    return nc


_CACHE = {}


def pack_inputs(x, w1, w2, wr):
    """Pack one batch's x plus the three weights into [128, nt+3, 128]."""
    n, d = x.shape
    nt = n // P
    pk = np.empty((P, nt + 3, d), dtype=np.float32)
    pk[:, 0:nt, :] = x.reshape(P, nt, d)
    pk[:, nt + 0, :] = wr
    pk[:, nt + 1, :] = w1
    pk[:, nt + 2, :] = w2
    return np.ascontiguousarray(pk)


def kernel(x, w1, w2, wr):
    x = np.ascontiguousarray(np.asarray(x), dtype=np.float32)
    w1 = np.ascontiguousarray(np.asarray(w1), dtype=np.float32)
    w2 = np.ascontiguousarray(np.asarray(w2), dtype=np.float32)
    wr = np.ascontiguousarray(np.asarray(wr), dtype=np.float32)
    b, n, d = x.shape
    if "nc" not in _CACHE:
        _CACHE["nc"] = build_kernel(n)
    nc = _CACHE["nc"]
    in_maps = [{"xall": pack_inputs(x[i], w1, w2, wr)} for i in range(b)]
    res = run_bass_kernel_spmd(nc, in_maps, core_ids=list(range(b)))
    return np.stack([res.results[i]["out"][0] for i in range(b)])
